# revision 26
# baseline (speedup 1.0000x reference)
"""Trainium2 Bass kernel for nn_Critic (gnn_message_passing).

Pure data-parallel over batch: 8 cores x 128 rows.

Single-query attention is algebraically collapsed: score s[b,n] =
feat[b,n,:] . qk[b,:] with qk = ego' @ (Wq @ Wk^T * scale) (weight-only
product folded on host).  Pooling happens in feature space (14 dims)
before the Wv projection; the subject-bus id subtraction contributes a
rank-1 term via an extra contraction row with host-folded weights
-(Wv[0]+Wv[7]).

BatchNorm batch stats are carried by per-head 16x16 Gram matrices
Gt = pool~^T pool~ (pool~ = normalized pooled features, +subj term,
+ones column): sum_b x[:,v] = Gt[:,15] . wv[:,v] and sum_b x[:,v]^2 =
wv[:,v]^T Gt wv[:,v].  Each head's Gram matrix is AllReduced the moment
its pooling finishes, so the three tiny collectives pipeline behind the
remaining heads' elementwise work.  A dummy 4-byte AllReduce issued at
kernel start absorbs the cross-core NEFF-launch skew so the real
collectives rendezvous without waiting.

Per-feature sums come from two batched K=45 matmuls against host-packed
stacked layouts; 1/sqrt(var+eps) = exp(-0.5*ln(var+eps)) on a single
fused [128,6] tile.  The BN affine is folded into the head inputs
(x' = s3*x) and biases (b1' = b1 + t0@W1); elu(x)+1 = relu(x) +
min(exp(x),1) with the -1 folded into one host-side output constant.
Head matmuls run in bf16 (weights shipped bf16 alongside the bf16
feature planes in one DMA).
"""

import os
import numpy as np
from contextlib import ExitStack

import ml_dtypes
import concourse.bacc as bacc
import concourse.tile as tile
from concourse import mybir
import concourse.bass as bass
from concourse.bass_utils import run_bass_kernel_spmd
from concourse.masks import make_identity

B, N, V = 1024, 256, 200
NC = 8
BS = B // NC  # 128 rows per core
F32 = mybir.dt.float32
BF16 = mybir.dt.bfloat16
ALU = mybir.AluOpType
ACTF = mybir.ActivationFunctionType
SCALE = float(1.0 / np.sqrt(V))
NEG = -1.0e9

VC = [(0, 128), (128, 200)]  # v-dim chunks

# tall weight tensor column layout ([128, C_TALL] f32; 200-row weights
# packed as chunk0 rows 0:128 | chunk1 rows 0:72)
W1 = 0        # 2 x 600 (t_W1 heads concatenated: k*200+v)
EW2 = 1200    # 2 x 200
W2 = 1600     # 2 x 3
B1T = 1606    # 2 x 3
EW3 = 1612    # 2 x 1
EB1 = 1614    # 2 x 1
EB2 = 1616    # 2 x 1
GAM = 1618    # 2 x 1
BET = 1620    # 2 x 1
C_TALL = 1622

# short weight tensor ([96, C_SHORT] f32); stacked regions use 32-aligned
# segment blocks (engine SBUF access must start at partition 0/32/64/96)
WV = 0        # 600: rows 0:14 Wv concat, row14 = -(Wv[0]+Wv[7]), row15 = 0
WC = 600      # 35: rows 0:6 = (Wq @ Wk^T)[1:7] * scale, segs u|d|p
EW1 = 635     # 200: rows 0:4
CB = 835      # 1: row 0 = sum(t_b2) + e_b3 - sum(t_W2)
WVS = 836     # 200: wv stacked: rows 32s:32s+15 = wv seg s, zeros elsewhere
OB = 1036     # 3: block-ones: rows 32s:32s+15 of col s = 1, zeros elsewhere
C_SHORT = 1039

# bf16 tensor column layout ([128, C_BF] bf16)
PL = 0        # 14 x 256 feature planes
W1B = 14 * N  # 1200: w1 in bf16, chunked like tall W1
C_BF = W1B + 1200

_cache = {}


def build_nc():
    STAGE = int(os.environ.get("K_STAGE", "9"))
    NO_CC = bool(os.environ.get("NO_CC"))
    nc = bacc.Bacc(None)

    lf = nc.dram_tensor("lf", [BS, 2 * N + 1], F32, kind="ExternalInput")
    mp = nc.dram_tensor("mp", [BS, C_BF], BF16, kind="ExternalInput")
    tallt = nc.dram_tensor("tall", [128, C_TALL], F32, kind="ExternalInput")
    shortt = nc.dram_tensor("short", [96, C_SHORT], F32, kind="ExternalInput")
    egoT = nc.dram_tensor("egoT", [10, BS], F32, kind="ExternalInput")
    out = nc.dram_tensor("out", [BS, 1], F32, kind="ExternalOutput")

    SEG = [('u', 14, 0), ('d', 14, 14), ('p', 7, 28)]
    SI = {'u': 0, 'd': 1, 'p': 2}

    with tile.TileContext(nc) as tc:
        with ExitStack() as ctx:
            sb = ctx.enter_context(tc.tile_pool(name="sb", bufs=1))
            ps_sm = ctx.enter_context(tc.tile_pool(name="ps_sm", bufs=2, space="PSUM"))
            ps_big = ctx.enter_context(tc.tile_pool(name="ps_big", bufs=3, space="PSUM"))
            psg = ctx.enter_context(tc.tile_pool(name="psg", bufs=1, space="PSUM"))
            dram = ctx.enter_context(tc.tile_pool(name="dram", bufs=1, space="DRAM"))

            # ---------------- dummy collective: absorb NEFF-launch skew ----------------
            dumm = sb.tile([1, 1], F32)
            nc.gpsimd.memset(dumm, 1.0)
            in_d = dram.tile([1, 1], F32, name="in_d")
            nc.sync.dma_start(out=in_d[:], in_=dumm)
            out_d = dram.tile([1, 1], F32, addr_space="Shared", name="out_d")
            if not NO_CC:
                nc.gpsimd.collective_compute(
                    "AllReduce", ALU.add, ins=[in_d[:]], outs=[out_d[:]],
                    replica_groups=[list(range(NC))])

            # ---------------- DMA in (2 HWDGE queues) ----------------
            lf_sb = sb.tile([BS, 2 * N + 1], F32)
            nc.sync.dma_start(out=lf_sb, in_=lf[:])
            mp_sb = sb.tile([BS, C_BF], BF16)
            nc.sync.dma_start(out=mp_sb, in_=mp[:])
            short = sb.tile([96, C_SHORT], F32)
            nc.scalar.dma_start(out=short, in_=shortt[:])
            ego_sb = sb.tile([6, BS], F32)
            nc.scalar.dma_start(out=ego_sb, in_=egoT[0:6, :])
            egoM_sb = sb.tile([4, BS], F32)
            nc.scalar.dma_start(out=egoM_sb, in_=egoT[6:10, :])
            tall = sb.tile([128, C_TALL], F32)
            nc.scalar.dma_start(out=tall, in_=tallt[:])

            plane = lambda f: mp_sb[:, f * N:(f + 1) * N]
            loc = lf_sb[:, 0:N]
            flagc = lf_sb[:, N:2 * N]
            subj_id = lf_sb[:, 2 * N:2 * N + 1]
            subj_loc = lf_sb[:, 0:1]

            # ---------------- constants (GpSimd, off critical path) ----------------
            ones_col = sb.tile([128, 1], F32)
            nc.gpsimd.memset(ones_col, 1.0)
            ones_row = sb.tile([1, 128], F32)
            nc.gpsimd.memset(ones_row, 1.0)
            eps_col = sb.tile([128, 1], F32)
            nc.gpsimd.memset(eps_col, 1.0e-5)
            prod_stack = sb.tile([96, V], F32)
            nc.gpsimd.memset(prod_stack, 0.0)
            ident = sb.tile([128, 128], F32)
            make_identity(nc, ident)

            # ---------------- masks -> score accumulators (DVE, bf16) ----------------
            acc = {}
            geM = sb.tile([BS, N], BF16)
            nc.vector.tensor_scalar(geM, loc, subj_loc, NEG, op0=ALU.is_ge, op1=ALU.mult)
            nfM = sb.tile([BS, N], BF16)
            nc.vector.tensor_scalar(nfM, flagc, 1.0e9, NEG, op0=ALU.mult, op1=ALU.add)
            acc['u'] = sb.tile([BS, N], BF16, tag="accu", name="accu")
            nc.vector.tensor_tensor(acc['u'], geM, nfM, op=ALU.min)
            acc['p'] = sb.tile([BS, N], BF16, tag="accp", name="accp")
            nc.vector.tensor_scalar(acc['p'], flagc, NEG, None, op0=ALU.mult)
            leM = sb.tile([BS, N], BF16)
            nc.vector.tensor_scalar(leM, loc, subj_loc, NEG, op0=ALU.is_le, op1=ALU.mult)
            acc['d'] = sb.tile([BS, N], BF16, tag="accd", name="accd")
            nc.vector.tensor_tensor(acc['d'], leM, nfM, op=ALU.min)

            # ---------------- qk (PE; copy via DVE to keep Scalar free) ----------------
            qk_ps = ps_sm.tile([BS, 35], F32, tag="sm", name="qk_ps")
            nc.tensor.matmul(qk_ps, ego_sb, short[0:6, WC:WC + 35],
                             start=True, stop=True)
            qk_sb = sb.tile([BS, 35], F32)
            nc.vector.tensor_copy(qk_sb, qk_ps)

            # ---------------- ego MLP early (PE otherwise idle) ----------------
            q1T, q2T = [], []
            for j, (w0, w1c) in enumerate(VC):
                pc = w1c - w0
                qp = ps_big.tile([pc, BS], F32, tag="big", name="qp")
                nc.tensor.matmul(qp, short[0:4, EW1 + w0:EW1 + w1c], egoM_sb,
                                 start=True, stop=True)
                qs = sb.tile([pc, BS], F32, tag=f"q1T{j}", name=f"q1T{j}")
                nc.scalar.activation(qs, qp, ACTF.Relu,
                                     bias=tall[0:pc, EB1 + j:EB1 + j + 1], scale=1.0)
                q1T.append(qs)
            for j, (w0, w1c) in enumerate(VC):
                pc = w1c - w0
                qp = ps_big.tile([pc, BS], F32, tag="big", name="qp2")
                for i, (c0, c1) in enumerate(VC):
                    nc.tensor.matmul(qp, tall[0:c1 - c0, EW2 + i * 200 + w0:EW2 + i * 200 + w1c],
                                     q1T[i], start=(i == 0), stop=(i == 1))
                qs = sb.tile([pc, BS], F32, tag=f"q2T{j}", name=f"q2T{j}")
                nc.scalar.activation(qs, qp, ACTF.Relu,
                                     bias=tall[0:pc, EB2 + j:EB2 + j + 1], scale=1.0)
                q2T.append(qs)
            G = psg.tile([BS, 1], F32)
            for i, (c0, c1) in enumerate(VC):
                nc.tensor.matmul(G, q2T[i], tall[0:c1 - c0, EW3 + i:EW3 + i + 1],
                                 start=(i == 0), stop=False, skip_group_check=True)
            nc.tensor.matmul(G, ones_row, short[0:1, CB:CB + 1], start=False,
                             stop=False, skip_group_check=True)

            # ---------------- per-segment pipeline: scores -> softmax -> pool
            # -> Gram -> collective; transposes/x^T fill the gaps ----------------
            scr_v = sb.tile([BS, N], BF16)
            pool, w_t, rs_t, wsum1 = {}, {}, {}, {}
            in_b, out_b, poolT = {}, {}, {}
            xT = {}
            for s, nf, j0 in SEG:
                si = SI[s]
                # scores (DVE STT chain, bf16)
                for f in range(nf):
                    nc.vector.scalar_tensor_tensor(
                        acc[s], plane(f), qk_sb[:, j0 + f:j0 + f + 1], acc[s],
                        op0=ALU.mult, op1=ALU.add)
                # softmax pieces
                w_t[s] = sb.tile([BS, N], BF16, tag=f"w{s}", name=f"w{s}")
                se = sb.tile([BS, 1], F32, tag=f"se{s}", name=f"se{s}")
                nc.scalar.activation(w_t[s], acc[s], ACTF.Exp, bias=0.0, scale=1.0,
                                     accum_out=se)
                seb = sb.tile([BS, 1], F32, tag=f"seb{s}", name=f"seb{s}")
                nc.vector.tensor_scalar_add(seb, se, 1.0e-30)
                rs_t[s] = sb.tile([BS, 1], F32, tag=f"rs{s}", name=f"rs{s}")
                nc.vector.reciprocal(rs_t[s], seb)
                wsum1[s] = sb.tile([BS, 1], F32, tag=f"ws{s}", name=f"ws{s}")
                nc.vector.tensor_tensor(wsum1[s], se, rs_t[s], op=ALU.mult)
                # pool (DVE STT + f32 accum)
                pool[s] = sb.tile([BS, 16], F32, tag=f"pool{s}", name=f"pool{s}")
                for f in range(nf):
                    nc.vector.scalar_tensor_tensor(
                        scr_v, plane(f), 1.0, w_t[s], op0=ALU.mult, op1=ALU.mult,
                        accum_out=pool[s][:, f:f + 1])
                nc.vector.tensor_scalar_mul(pool[s][:, 0:nf], pool[s][:, 0:nf], rs_t[s])
                if nf < 14:
                    nc.vector.memset(pool[s][:, nf:14], 0.0)
                nc.vector.tensor_tensor(pool[s][:, 14:15], subj_id, wsum1[s], op=ALU.mult)
                nc.vector.memset(pool[s][:, 15:16], 1.0)
                # Gram matrix -> DRAM -> collective
                gt_ps = ps_sm.tile([16, 16], F32, tag="sm", name=f"gt{s}")
                nc.tensor.matmul(gt_ps, pool[s], pool[s], start=True, stop=True)
                gt_sb = sb.tile([16, 16], F32, tag=f"gts{s}", name=f"gts{s}")
                nc.vector.tensor_copy(gt_sb, gt_ps)
                in_b[s] = dram.tile([16, 16], F32, name=f"inb{s}")
                nc.sync.dma_start(out=in_b[s][:], in_=gt_sb)
                out_b[s] = dram.tile([16, 16], F32, addr_space="Shared", name=f"outb{s}")
                if NO_CC:
                    nc.sync.dma_start(out=out_b[s][:], in_=in_b[s][:])
                else:
                    nc.gpsimd.collective_compute(
                        "AllReduce", ALU.add, ins=[in_b[s][:]], outs=[out_b[s][:]],
                        replica_groups=[list(range(NC))])
                # overlap: pool^T and x^T for this segment (indep of collective)
                pT = ps_sm.tile([16, BS], F32, tag="sm", name=f"pT{s}")
                nc.tensor.transpose(pT, pool[s], ident)
                poolT[s] = sb.tile([16, BS], F32, tag=f"pTs{s}", name=f"pTs{s}")
                nc.scalar.activation(poolT[s], pT, ACTF.Copy, bias=0.0, scale=1.0)
                xT[s] = []
                for i, (c0, c1) in enumerate(VC):
                    xps = ps_big.tile([c1 - c0, BS], F32, tag="big", name="xps")
                    nc.tensor.matmul(xps, short[0:15, WV + si * V + c0:WV + si * V + c1],
                                     poolT[s][0:15, :], start=True, stop=True)
                    xsb = sb.tile([c1 - c0, BS], F32, tag=f"xT{s}{i}", name=f"xT{s}{i}")
                    nc.scalar.activation(xsb, xps, ACTF.Copy, bias=0.0, scale=1.0)
                    xT[s].append(xsb)

            if STAGE <= 3:
                g_sb = sb.tile([BS, 1], F32, name="g_sb")
                nc.vector.tensor_copy(g_sb, pool['u'][:, 0:1])
                nc.sync.dma_start(out=out[:], in_=g_sb)
                return nc

            # ---------------- collective readbacks (SWDGE; GpSimd queue idle) ----------------
            gg = {}
            for s, nf, j0 in SEG:
                gg[s] = sb.tile([16, 16], F32, tag=f"gg{s}", name=f"gg{s}")
                nc.gpsimd.dma_start(out=gg[s], in_=out_b[s][:])

            # ---------------- per-feature stats: T1 + stacked prod ----------------
            for s, nf, j0 in SEG:
                si = SI[s]
                t1 = ps_big.tile([16, V], F32, tag="big", name=f"t1{s}")
                nc.tensor.matmul(t1, gg[s][0:15, :],
                                 short[0:15, WV + si * V:WV + (si + 1) * V],
                                 start=True, stop=True)
                nc.vector.tensor_tensor(prod_stack[32 * si:32 * si + 15, :],
                                        short[0:15, WV + si * V:WV + (si + 1) * V],
                                        t1[0:15, :], op=ALU.mult)

            # column sums: skinny sum matmuls + one batched ssq matmul per chunk
            var_all = sb.tile([128, 6], F32)  # cols 0:3 chunk0, 3:6 chunk1(rows 0:72)
            st_t = []
            for j, (c0, c1) in enumerate(VC):
                pc = c1 - c0
                stp = ps_sm.tile([pc, 6], F32, tag="sm", name=f"stp{j}")
                for s, nf, j0 in SEG:
                    si = SI[s]
                    nc.tensor.matmul(stp[:, si:si + 1],
                                     short[0:15, WV + si * V + c0:WV + si * V + c1],
                                     gg[s][0:15, 15:16], start=True, stop=True)
                nc.tensor.matmul(stp[:, 3:6], prod_stack[:, c0:c1],
                                 short[0:96, OB:OB + 3], start=True, stop=True)
                st = sb.tile([pc, 6], F32, tag=f"st{j}", name=f"st{j}")
                nc.vector.tensor_scalar(st, stp, 1.0 / B, None, op0=ALU.mult)
                sq = sb.tile([pc, 3], F32, tag=f"sq{j}", name=f"sq{j}")
                nc.vector.tensor_tensor(sq, st[:, 0:3], st[:, 0:3], op=ALU.mult)
                nc.vector.tensor_tensor(var_all[0:pc, 3 * j:3 * j + 3],
                                        st[:, 3:6], sq, op=ALU.subtract)
                st_t.append(st)
            # rstd = exp(-0.5*ln(var+eps)), one fused tile (stays in exp/ln set)
            lnv = sb.tile([128, 6], F32)
            nc.scalar.activation(lnv, var_all, ACTF.Ln, bias=eps_col, scale=1.0)
            rstd = sb.tile([128, 6], F32)
            nc.scalar.activation(rstd, lnv, ACTF.Exp, bias=0.0, scale=-0.5)

            s3_t, t03b_t = [], []
            for j, (c0, c1) in enumerate(VC):
                pc = c1 - c0
                gam_b = tall[0:pc, GAM + j:GAM + j + 1]
                gam_b = bass.AP(tensor=gam_b.tensor, offset=gam_b.offset,
                                ap=[gam_b.ap[0], [0, 3]])
                bet_b = tall[0:pc, BET + j:BET + j + 1]
                bet_b = bass.AP(tensor=bet_b.tensor, offset=bet_b.offset,
                                ap=[bet_b.ap[0], [0, 3]])
                s3 = sb.tile([pc, 3], F32, tag=f"s3{j}", name=f"s3{j}")
                nc.vector.tensor_tensor(s3, rstd[0:pc, 3 * j:3 * j + 3], gam_b,
                                        op=ALU.mult)
                z3 = sb.tile([pc, 3], F32, tag=f"z3{j}", name=f"z3{j}")
                nc.vector.tensor_tensor(z3, st_t[j][:, 0:3], s3, op=ALU.mult)
                t03b = sb.tile([pc, 3], BF16, tag=f"t03{j}", name=f"t03{j}")
                nc.vector.tensor_tensor(t03b, bet_b, z3, op=ALU.subtract)
                s3_t.append(s3)
                t03b_t.append(t03b)

            if STAGE <= 5:
                g_sb = sb.tile([BS, 1], F32, name="g_sb")
                nc.vector.tensor_copy(g_sb, s3_t[0][:, 0:1])
                nc.sync.dma_start(out=out[:], in_=g_sb)
                return nc

            # ---------------- BN folded into head inputs (bf16) ----------------
            xs = {}  # xs[k][i] = s3 * xT, bf16
            for k, s in enumerate(['u', 'd', 'p']):
                xs[k] = []
                for i, (c0, c1) in enumerate(VC):
                    t = sb.tile([c1 - c0, BS], BF16, tag=f"xs{k}{i}", name=f"xs{k}{i}")
                    nc.vector.tensor_scalar_mul(t, xT[s][i], s3_t[i][:, k:k + 1])
                    xs[k].append(t)
            # b1' = b1 + t0 @ W1 (bf16 matmuls, N=3 batched over heads? per k col)
            B1 = []
            for j, (w0, w1c) in enumerate(VC):
                pc = w1c - w0
                bt = sb.tile([pc, 3], F32, tag=f"B1{j}", name=f"B1{j}")
                for k in range(3):
                    bp = ps_sm.tile([pc, 1], F32, tag="sm", name="bp")
                    for i, (c0, c1) in enumerate(VC):
                        nc.tensor.matmul(
                            bp,
                            mp_sb[0:c1 - c0, W1B + i * 600 + k * V + w0:W1B + i * 600 + k * V + w1c],
                            t03b_t[i][:, k:k + 1],
                            start=(i == 0), stop=(i == 1))
                    nc.vector.tensor_copy(bt[:, k:k + 1], bp)
                nc.vector.tensor_tensor(bt, bt, tall[0:pc, B1T + j * 3:B1T + j * 3 + 3],
                                        op=ALU.add)
                B1.append(bt)

            # ---------------- heads: elu+1 folded, accumulate into G ----------------
            for k in range(3):
                for j, (w0, w1c) in enumerate(VC):
                    pc = w1c - w0
                    hp = ps_big.tile([pc, BS], F32, tag="big", name="hp")
                    for i, (c0, c1) in enumerate(VC):
                        nc.tensor.matmul(
                            hp,
                            mp_sb[0:c1 - c0, W1B + i * 600 + k * V + w0:W1B + i * 600 + k * V + w1c],
                            xs[k][i], start=(i == 0), stop=(i == 1))
                    eh = sb.tile([pc, BS], F32, tag=f"eh{j}", name=f"eh{j}")
                    nc.scalar.activation(eh, hp, ACTF.Exp, bias=B1[j][:, k:k + 1],
                                         scale=1.0)
                    rh = sb.tile([pc, BS], F32, tag=f"rh{j}", name=f"rh{j}")
                    nc.vector.tensor_scalar(rh, hp, B1[j][:, k:k + 1], 0.0,
                                            op0=ALU.add, op1=ALU.max)
                    ht = sb.tile([pc, BS], F32, tag=f"ht{j}", name=f"ht{j}")
                    nc.vector.scalar_tensor_tensor(ht, eh, 1.0, rh,
                                                   op0=ALU.min, op1=ALU.add)
                    nc.tensor.matmul(G, ht, tall[0:pc, W2 + j * 3 + k:W2 + j * 3 + k + 1],
                                     start=False, stop=(k == 2 and j == 1),
                                     skip_group_check=True)

            g_sb = sb.tile([BS, 1], F32)
            nc.vector.tensor_copy(g_sb, G)
            nc.sync.dma_start(out=out[:], in_=g_sb)

    nc.finalize()
    return nc


def prep_inputs(inputs):
    """Host-side prep: layout/dtype for data, constant folding for weights."""
    m = np.ascontiguousarray(inputs["merged"], dtype=np.float32)
    a = np.ascontiguousarray(inputs["a"], dtype=np.float32)
    f32 = lambda x: np.ascontiguousarray(x, dtype=np.float32)

    up_Wq, up_Wk, up_Wv = inputs["up_Wq"], inputs["up_Wk"], inputs["up_Wv"]
    dn_Wq, dn_Wk, dn_Wv = inputs["dn_Wq"], inputs["dn_Wk"], inputs["dn_Wv"]
    pv_Wq, pv_Wk, pv_Wv = inputs["pv_Wq"], inputs["pv_Wk"], inputs["pv_Wv"]
    t_W1, t_b1, t_W2, t_b2 = inputs["t_W1"], inputs["t_b1"], inputs["t_W2"], inputs["t_b2"]
    e_W1, e_b1, e_W2, e_b2 = inputs["e_W1"], inputs["e_b1"], inputs["e_W2"], inputs["e_b2"]
    e_W3, e_b3 = inputs["e_W3"], inputs["e_b3"]
    gamma, beta = inputs["gamma"], inputs["beta"]

    # wc = (Wq @ Wk^T)[1:7] * scale per segment (weight-only constant)
    wc = np.zeros((6, 35), np.float32)
    for (Wq, Wk), o0, nf in [((up_Wq, up_Wk), 0, 14), ((dn_Wq, dn_Wk), 14, 14),
                             ((pv_Wq, pv_Wk), 28, 7)]:
        wc[:, o0:o0 + nf] = (np.asarray(Wq, np.float32) @ np.asarray(Wk, np.float32).T)[1:7] * SCALE
    # wv with pv zero-padded to 14 rows, + ext row 14 = -(row0+row7)
    wv = np.zeros((16, 3 * V), np.float32)
    wv[0:14, 0:V] = up_Wv
    wv[0:14, V:2 * V] = dn_Wv
    wv[0:7, 2 * V:3 * V] = pv_Wv
    wv[14] = -(wv[0] + wv[7])

    def chunk2(arr):  # [200, c] -> [128, 2c]
        arr = np.asarray(arr, np.float32)
        c = arr.shape[1]
        o = np.zeros((128, 2 * c), np.float32)
        o[:, :c] = arr[0:128]
        o[0:72, c:] = arr[128:200]
        return o

    w1 = np.concatenate([t_W1[0], t_W1[1], t_W1[2]], axis=1)          # [200,600]
    w1c = chunk2(w1)
    tall = np.concatenate([
        w1c, chunk2(e_W2), chunk2(np.asarray(t_W2)[:, :, 0].T),
        chunk2(np.asarray(t_b1).T), chunk2(e_W3),
        chunk2(np.asarray(e_b1)[:, None]), chunk2(np.asarray(e_b2)[:, None]),
        chunk2(np.asarray(gamma)[:, None]), chunk2(np.asarray(beta)[:, None]),
    ], axis=1)
    assert tall.shape == (128, C_TALL), tall.shape

    short = np.zeros((96, C_SHORT), np.float32)
    short[0:16, WV:WV + 3 * V] = wv
    short[0:6, WC:WC + 35] = wc
    short[0:4, EW1:EW1 + V] = e_W1
    short[0, CB] = float(np.sum(t_b2)) + float(np.sum(e_b3)) - float(np.sum(t_W2))
    for si in range(3):  # 32-aligned stacked blocks for batched column sums
        short[32 * si:32 * si + 15, WVS:WVS + V] = wv[0:15, si * V:(si + 1) * V]
        short[32 * si:32 * si + 15, OB + si] = 1.0

    # data: loc/flag/subj_id f32; feature planes + w1 copy in bf16
    lf_full = np.concatenate([m[:, :, 2], m[:, :, 14], m[:, 0, 0:1]], axis=1)
    feats = m[:, :, 0:14].transpose(0, 2, 1).reshape(B, 14 * N)
    w1b = w1c.astype(ml_dtypes.bfloat16)

    in_maps = []
    for c in range(NC):
        sl = slice(c * BS, (c + 1) * BS)
        sh = m[sl]
        egoT = np.zeros((10, BS), np.float32)
        egoT[0:5] = sh[:, 0, 1:6].T
        egoT[5] = a[sl]
        egoT[6:9] = sh[:, 0, 3:6].T
        egoT[9] = a[sl]
        mpc = np.empty((BS, C_BF), dtype=ml_dtypes.bfloat16)
        mpc[:, 0:14 * N] = feats[sl]
        mpc[:, W1B:] = w1b
        in_maps.append(dict(
            lf=f32(lf_full[sl]), mp=mpc,
            tall=tall, short=short, egoT=f32(egoT)))
    return in_maps


def _build():
    nc = build_nc()
    if not nc.is_finalized():
        nc.finalize()
    return nc


def kernel(**inputs):
    if "nc" not in _cache:
        _cache["nc"] = _build()
    nc = _cache["nc"]
    in_maps = prep_inputs(inputs)
    r = run_bass_kernel_spmd(nc, in_maps, list(range(NC)), trace=False)
    _cache["last"] = r
    out = np.concatenate([r.results[c]["out"] for c in range(NC)], axis=0)
    return out.reshape(-1, 1).astype(np.float32)
